# revision 31
# baseline (speedup 1.0000x reference)
"""MoE top-2 (2 experts) FFN kernel for TRN2, 8 NeuronCores.

Problem (hardcoded):
  x:   (8192, 2048) f32 tokens
  two expert FFNs: d_model=2048 -> d_ff=8192 (gelu exact) -> 2048
  out[i] = w0[i] * FFN0(x[i]) + w1[i] * FFN1(x[i])
  where w_e[i] = sum of top2_weight[i, s] over slots s with (top2_exp_id[i,s] % 2) == e

Strategy:
  - Host: fold top-2 gating into per-token scalars w0/w1; transpose x;
    gather each expert's active tokens (those with w_e > 0, ~75% of
    tokens) and split them evenly across the 8 cores. Capacities are
    sized at runtime to exactly ceil(|S_e|/8) (rounded to a multiple of
    4), so there is no fixed-capacity padding waste and no overflow
    fallback.
  - Data-parallel over gathered tokens: every core carries ~1/8 of each
    expert's active set.
  - On-core: activations kept transposed ([d_model|d_ff on partitions] x
    [tokens on free dim]) so both matmul layers contract along
    partitions with weights in their natural HBM layout.
  - bf16 weights + activations (fp32 PSUM accumulate): same 1 cycle/row
    PE rate as fp32r but half the DMA/SBUF traffic, and avoids the
    fp32r per-matmul pipeline overhead. rel-err ~4e-3, well inside the
    2e-2 gate.
  - Both experts' gathered xT tiles stay resident in SBUF (bf16 makes
    them fit); expert 1's tiles prefetch during expert 0's compute so
    the expert transition does not stall on DMA.
  - d_ff processed in chunks of 512; layer-2 partials accumulated into
    an SBUF-resident y so each weight byte is streamed exactly once.
  - Software-pipelined emission: PE order L1(0),L1(1),L2(0),L1(2),L2(1)
    ... so gelu/gate (ACT+DVE) of chunk i overlaps L1(i+1) matmuls.
"""

import os

import numpy as np
import ml_dtypes

import concourse.bass as bass
import concourse.mybir as mybir
import concourse.tile as tile
from concourse import bacc
from concourse import bass_utils


def _ensure_ntff_hook():
    """This image's `antenv` lacks `axon_hooks`, so boot-time NTFF hook
    install degrades silently and trace=True captures nothing. Register a
    shim module and install the ctypes-driven hook (same as trn_boot)."""
    import sys
    import types

    if "antenv.axon_hooks" in sys.modules:
        return
    mod = types.ModuleType("antenv.axon_hooks")
    mod._hook = None

    def set_axon_ntff_profile_hook(h):
        mod._hook = h

    def get_axon_ntff_profile_hook():
        return mod._hook

    mod.set_axon_ntff_profile_hook = set_axon_ntff_profile_hook
    mod.get_axon_ntff_profile_hook = get_axon_ntff_profile_hook
    sys.modules["antenv.axon_hooks"] = mod
    try:
        from trn_agent_boot.trn_boot import _ntff_profile_via_ctypes

        hook = _ntff_profile_via_ctypes("/opt/axon/libaxon_pjrt.so")
        if hook is not None:
            mod._hook = hook
    except Exception:
        pass


P = 128
D_MODEL = 2048
D_FF = 8192
N_LOCAL = 8192
N_CORES = 8
KM = D_MODEL // P              # 16 contraction tiles for layer 1
CHUNK = 512                    # d_ff chunk held in PSUM per pass
FC = CHUNK // P                # 4 d_ff tiles per chunk
NCHUNK = D_FF // CHUNK         # 16
M2 = D_MODEL // P              # 16 output d_model tiles

F32 = mybir.dt.float32
BF16 = mybir.dt.bfloat16
BF16_NP = ml_dtypes.bfloat16
GELU = mybir.ActivationFunctionType.Gelu


def _blocks(total):
    """Moving-dim blocks, each <= 512 (one PSUM bank of fp32), near-equal
    and even-sized so every block's byte offset stays 4B-aligned."""
    n = (total + 511) // 512
    pairs = total // 2
    base = pairs // n
    out = []
    off = 0
    for i in range(n):
        hp = base + (1 if i < pairs - base * n else 0)
        hs = hp * 2
        out.append((off, hs))
        off += hs
    assert off == total and all(2 <= hs <= 512 for _, hs in out)
    return out


def _build_sparse(nc, caps):
    """Per-expert gathered tokens (caps[e] per core); expert passes run
    back-to-back, with expert 1's xT prefetched during expert 0."""
    HS = [_blocks(caps[e]) for e in range(2)]
    capmax = max(caps)
    xg = [
        nc.dram_tensor(f"xg{e}", (D_MODEL, caps[e]), BF16, kind="ExternalInput").ap()
        for e in range(2)
    ]
    # w1 is host-packed to (c, k) -> contiguous [P, CHUNK] blocks so each
    # strip DMA is one linear 128KB read (column slices of the natural
    # row-major layout have 1KB lines / 16KB stride and run ~3x slower)
    w1 = [
        nc.dram_tensor(
            f"w1p_{e}", (NCHUNK * KM, P, CHUNK), BF16, kind="ExternalInput"
        ).ap()
        for e in range(2)
    ]
    w2 = [
        nc.dram_tensor(f"w2_{e}", (D_FF, D_MODEL), BF16, kind="ExternalInput").ap()
        for e in range(2)
    ]
    b1t = [
        nc.dram_tensor(f"b1t_{e}", (P, D_FF // P), F32, kind="ExternalInput").ap()
        for e in range(2)
    ]
    b2t = [
        nc.dram_tensor(f"b2t_{e}", (P, M2), F32, kind="ExternalInput").ap()
        for e in range(2)
    ]
    wgg = [
        nc.dram_tensor(f"wgg{e}", (P, caps[e]), BF16, kind="ExternalInput").ap()
        for e in range(2)
    ]
    yt = [
        nc.dram_tensor(f"yt{e}", (D_MODEL, caps[e]), F32, kind="ExternalOutput").ap()
        for e in range(2)
    ]

    with tile.TileContext(nc) as tc:
        with (
            tc.tile_pool(name="const", bufs=1) as const_pool,
            tc.tile_pool(name="w1s", bufs=8) as w1_pool,
            tc.tile_pool(name="w2s", bufs=8) as w2_pool,
            tc.tile_pool(name="ht", bufs=8) as ht_pool,
            tc.tile_pool(name="ps", bufs=8, space="PSUM") as psum_pool,
        ):
            # Both experts' xT k-tiles stay resident: expert 1's tiles
            # are prefetched while expert 0 computes.
            xt_sb = [
                [
                    const_pool.tile(
                        [P, caps[e]], BF16, tag=f"xt{e}_{k}", name=f"xt_sb{e}_{k}"
                    )
                    for k in range(KM)
                ]
                for e in range(2)
            ]
            y_sb = const_pool.tile([P, M2, capmax], F32, tag="y", name="y_sb")
            wgg_sb = [
                const_pool.tile([P, caps[e]], BF16, tag=f"wgg{e}", name=f"wgg{e}_sb")
                for e in range(2)
            ]
            b1t_sb = [
                const_pool.tile([P, D_FF // P], F32, tag=f"b1t{e}", name=f"b1t{e}_sb")
                for e in range(2)
            ]
            b2t_sb = [
                const_pool.tile([P, M2], F32, tag=f"b2t{e}", name=f"b2t{e}_sb")
                for e in range(2)
            ]

            xg3 = [xg[e].rearrange("(ko p) t -> p ko t", p=P) for e in range(2)]
            yt3 = [yt[e].rearrange("(mo p) t -> p mo t", p=P) for e in range(2)]

            pairs = [(e, c) for e in range(2) for c in range(NCHUNK)]

            def emit_l1(e, c, first=False):
                """PE: layer-1 matmuls for one (expert, chunk)."""
                cap = caps[e]
                psums = [
                    [
                        psum_pool.tile(
                            [P, hs], F32, tag="ps", name=f"ps1_{e}_{c}_{f}_{h}"
                        )
                        for h, (off, hs) in enumerate(HS[e])
                    ]
                    for f in range(FC)
                ]
                for k in range(KM):
                    if e == 0 and c == 0:
                        # Startup is bound by per-DMA-queue delivery
                        # (measured ~140 sync / ~130 gpsimd / ~50
                        # scalar GB/s); balance the 5.2MB the first
                        # chunk needs across all three so every k-tile
                        # lands just ahead of the PE's k-march. k0 is
                        # split at the h-block boundary so the very
                        # first matmul's operand arrives first.
                        if k == 0:
                            for off, hs in HS[0]:
                                nc.sync.dma_start(
                                    xt_sb[0][0][:, off : off + hs],
                                    xg3[0][:, 0, off : off + hs],
                                )
                        else:
                            # the otherwise-idle gpsimd queue (~130
                            # GB/s) carries the whole xT stream so the
                            # fast sync queue stays short
                            nc.gpsimd.dma_start(xt_sb[0][k][:], xg3[0][:, k, :])
                        if k == KM - 1:
                            # consts on sync after its few w1 strips
                            # (nothing reads them before ~35us)
                            for ee in range(2):
                                nc.sync.dma_start(wgg_sb[ee][:], wgg[ee][:])
                                nc.sync.dma_start(b1t_sb[ee][:], b1t[ee][:])
                                nc.sync.dma_start(b2t_sb[ee][:], b2t[ee][:])
                    if e == 0 and 6 <= c <= 13 and k in (4, 12):
                        # prefetch expert-1's xT two k-tiles per chunk.
                        # Placed on sync BETWEEN its w1 strip pushes:
                        # sync's stream is self-paced by the w1 pool's
                        # slot WARs, so these actually fire at chunk-c
                        # time instead of racing ahead into the
                        # HBM-saturated startup window (gpsimd's queue
                        # has no such pacing and blasts everything
                        # immediately).
                        kk = (c - 6) * 2 + (0 if k == 4 else 1)
                        nc.sync.dma_start(xt_sb[1][kk][:], xg3[1][:, kk, :])
                    w1s = w1_pool.tile(
                        [P, CHUNK], BF16, tag="w1s", name=f"w1s_{e}_{c}_{k}"
                    )
                    if e == 0 and c == 0:
                        # with xT on gpsimd, sync and scalar split c0's
                        # w1 strips; sync (fast, short queue) then has
                        # room for the consts and all of c1's strips.
                        # c1 strips must NOT ride scalar: they'd be
                        # head-of-line blocked behind c0's ACTIVATEs
                        # in the engine FIFO.
                        w1_eng = nc.sync if k % 2 == 0 else nc.scalar
                    else:
                        w1_eng = nc.sync
                    w1_eng.dma_start(w1s[:], w1[e][c * KM + k])
                    for f in range(FC):
                        for h, (off, hs) in enumerate(HS[e]):
                            nc.tensor.matmul(
                                psums[f][h][:],
                                w1s[:, f * P : (f + 1) * P],
                                xt_sb[e][k][:, off : off + hs],
                                start=(k == 0),
                                stop=(k == KM - 1),
                            )
                return psums

            def emit_act(e, c, psums):
                """ACT+DVE: gelu(+b1), gate scale. Also W2 strip loads,
                and (on each expert's first chunk) the gated b2 y-init."""
                cap = caps[e]
                if c == 0:
                    for m in range(M2):
                        nc.vector.tensor_scalar_mul(
                            y_sb[:, m, :cap], wgg_sb[e][:], b2t_sb[e][:, m : m + 1]
                        )
                hts = []
                for f in range(FC):
                    ht = ht_pool.tile(
                        [P, capmax], BF16, tag="ht", name=f"ht_{e}_{c}_{f}"
                    )
                    col = c * FC + f
                    for h, (off, hs) in enumerate(HS[e]):
                        nc.scalar.activation(
                            ht[:, off : off + hs],
                            psums[f][h][:],
                            GELU,
                            bias=b1t_sb[e][:, col : col + 1],
                        )
                    nc.vector.tensor_mul(ht[:, :cap], ht[:, :cap], wgg_sb[e][:])
                    hts.append(ht)
                w2s = []
                # c0's w2 strips queue on scalar AFTER its ACTs (~40us,
                # needed ~58us) keeping them out of the contended
                # 25-35us window on sync/gpsimd
                w2_eng = nc.scalar if (e == 0 and c == 0) else nc.sync
                for f in range(FC):
                    w2f = w2_pool.tile(
                        [P, D_MODEL], BF16, tag="w2s", name=f"w2s_{e}_{c}_{f}"
                    )
                    row = (c * FC + f) * P
                    w2_eng.dma_start(w2f[:], w2[e][row : row + P, :])
                    w2s.append(w2f)
                return hts, w2s

            def emit_l2(e, c, hts, w2s):
                """PE: layer-2 matmuls; DVE: accumulate into y; store at
                the expert's last chunk."""
                cap = caps[e]
                for m in range(M2):
                    for h, (off, hs) in enumerate(HS[e]):
                        ps = psum_pool.tile(
                            [P, hs], F32, tag="ps", name=f"ps2_{e}_{c}_{m}_{h}"
                        )
                        for f in range(FC):
                            nc.tensor.matmul(
                                ps[:],
                                w2s[f][:, m * P : (m + 1) * P],
                                hts[f][:, off : off + hs],
                                start=(f == 0),
                                stop=(f == FC - 1),
                            )
                        ysl = y_sb[:, m, off : off + hs]
                        nc.vector.tensor_add(ysl, ysl, ps[:])
                        if c == NCHUNK - 1 and e == 1:
                            # final expert: store each h-slice as soon
                            # as it's done — halves the serial
                            # last-store on the kernel's exit path
                            nc.sync.dma_start(
                                yt3[e][:, m, off : off + hs],
                                y_sb[:, m, off : off + hs],
                            )
                    if c == NCHUNK - 1 and e == 0:
                        nc.sync.dma_start(yt3[e][:, m, :], y_sb[:, m, :cap])

            # PE warm-up: ~4.3us of dependency-free dummy matmuls run
            # while the startup DMAs stream in, so the HAM clock-gate
            # un-throttles (1.2 -> 2.4 GHz) before the first real
            # matmul. Without this the whole first chunk runs cold.
            # ~7us of dummy matmuls: the HAM clock-gate warms after
            # ~3.4us and stays warm (no idle gaps), while the startup
            # DMA burst (~5MB, HBM-bound) builds enough lead for chunk
            # 0 to then stream at the warm PE rate without stalls.
            warm_x = const_pool.tile([P, 512], BF16, tag="warm", name="warm_sb")
            nc.gpsimd.memset(warm_x[:], 0.0)
            for i in range(26):
                wps = psum_pool.tile([P, 512], F32, tag="ps", name=f"warm_ps{i}")
                nc.tensor.matmul(
                    wps[:], warm_x[:, :P], warm_x[:], start=True, stop=True
                )

            psums_cur = emit_l1(*pairs[0], first=True)
            for i, (e, c) in enumerate(pairs):
                hts, w2s = emit_act(e, c, psums_cur)
                if i + 1 < len(pairs):
                    psums_cur = emit_l1(*pairs[i + 1])
                emit_l2(e, c, hts, w2s)

    nc.compile()
    return nc


_CACHED = {}


def _get_nc(caps):
    key = tuple(caps)
    if key not in _CACHED:
        nc = bacc.Bacc(
            "TRN2",
            target_bir_lowering=False,
            debug=False,
            num_devices=N_CORES,
        )
        _CACHED[key] = _build_sparse(nc, caps)
    return _CACHED[key]


def _run(nc, in_maps):
    trace = bool(int(os.environ.get("KERNEL_TRACE", "0")))
    if trace:
        _ensure_ntff_hook()
    res = bass_utils.run_bass_kernel_spmd(
        nc, in_maps, core_ids=list(range(N_CORES)), trace=trace
    )
    if trace:
        kernel.last_exec_time_ns = res.exec_time_ns
        kernel.last_results = res
    return res


def kernel(**inputs):
    x = np.asarray(inputs["x_local"], dtype=np.float32)          # (8192, 2048)
    ids = np.asarray(inputs["top2_exp_id"])                       # (8192, 2)
    tw = np.asarray(inputs["top2_weight"], dtype=np.float32)      # (8192, 2)

    sel = (ids % 2).astype(np.float32)
    wge = [
        (tw * (1.0 - sel)).sum(axis=1).astype(np.float32),        # expert-0 gate
        (tw * sel).sum(axis=1).astype(np.float32),                # expert-1 gate
    ]

    xt = np.ascontiguousarray(x.T.astype(BF16_NP))                # (2048, 8192)

    shared = {}
    for e in range(2):
        w1bf = np.asarray(inputs[f"W1_{e}"], dtype=np.float32).astype(BF16_NP)
        # pack (D_MODEL, D_FF) -> (c*KM+k, P, CHUNK) contiguous blocks
        shared[f"w1p_{e}"] = np.ascontiguousarray(
            w1bf.reshape(KM, P, NCHUNK, CHUNK)
            .transpose(2, 0, 1, 3)
            .reshape(NCHUNK * KM, P, CHUNK)
        )
        shared[f"w2_{e}"] = np.ascontiguousarray(
            np.asarray(inputs[f"W2_{e}"], dtype=np.float32).astype(BF16_NP)
        )
        shared[f"b1t_{e}"] = np.ascontiguousarray(
            np.asarray(inputs[f"b1_{e}"], dtype=np.float32).reshape(D_FF // P, P).T
        )
        shared[f"b2t_{e}"] = np.ascontiguousarray(
            np.asarray(inputs[f"b2_{e}"], dtype=np.float32).reshape(M2, P).T
        )

    # Globally-balanced gathers: each expert's active set (~75% of all
    # tokens) is split evenly across the 8 cores; capacity is sized to
    # the actual max per-core load (multiple of 4), so padding waste is
    # at most 3 tokens per expert per core.
    glocs = [np.flatnonzero(wge[e] > 0) for e in range(2)]
    ceil = lambda a, b: -(-a // b)
    caps = [max(4, ceil(ceil(len(glocs[e]), N_CORES), 4) * 4) for e in range(2)]

    splits = [np.array_split(glocs[e], N_CORES) for e in range(2)]
    in_maps = []
    for c in range(N_CORES):
        m = dict(shared)
        for e in range(2):
            loc = splits[e][c]
            cnt = len(loc)
            xgc = np.zeros((D_MODEL, caps[e]), BF16_NP)
            xgc[:, :cnt] = xt[:, loc]
            m[f"xg{e}"] = xgc
            wggc = np.zeros((caps[e],), np.float32)
            wggc[:cnt] = wge[e][loc]
            m[f"wgg{e}"] = np.ascontiguousarray(
                np.broadcast_to(wggc.astype(BF16_NP), (P, caps[e]))
            )
        in_maps.append(m)

    res = _run(_get_nc(caps), in_maps)

    y = np.zeros((N_LOCAL, D_MODEL), np.float32)
    for c in range(N_CORES):
        for e in range(2):
            loc = splits[e][c]
            cnt = len(loc)
            y[loc] += res.results[c][f"yt{e}"].T[:cnt]
    return y


# revision 34
# speedup vs baseline: 1.0070x; 1.0070x over previous
"""MoE top-2 (2 experts) FFN kernel for TRN2, 8 NeuronCores.

Problem (hardcoded):
  x:   (8192, 2048) f32 tokens
  two expert FFNs: d_model=2048 -> d_ff=8192 (gelu exact) -> 2048
  out[i] = w0[i] * FFN0(x[i]) + w1[i] * FFN1(x[i])
  where w_e[i] = sum of top2_weight[i, s] over slots s with (top2_exp_id[i,s] % 2) == e

Strategy:
  - Host: fold top-2 gating into per-token scalars w0/w1; transpose x;
    gather each expert's active tokens (those with w_e > 0, ~75% of
    tokens) and split them evenly across the 8 cores. Capacities are
    sized at runtime to exactly ceil(|S_e|/8) (rounded to a multiple of
    4), so there is no fixed-capacity padding waste and no overflow
    fallback.
  - Data-parallel over gathered tokens: every core carries ~1/8 of each
    expert's active set.
  - On-core: activations kept transposed ([d_model|d_ff on partitions] x
    [tokens on free dim]) so both matmul layers contract along
    partitions with weights in their natural HBM layout.
  - bf16 weights + activations (fp32 PSUM accumulate): same 1 cycle/row
    PE rate as fp32r but half the DMA/SBUF traffic, and avoids the
    fp32r per-matmul pipeline overhead. rel-err ~4e-3, well inside the
    2e-2 gate.
  - Both experts' gathered xT tiles stay resident in SBUF (bf16 makes
    them fit); expert 1's tiles prefetch during expert 0's compute so
    the expert transition does not stall on DMA.
  - d_ff processed in chunks of 512; layer-2 partials accumulated into
    an SBUF-resident y so each weight byte is streamed exactly once.
  - Software-pipelined emission: PE order L1(0),L1(1),L2(0),L1(2),L2(1)
    ... so gelu/gate (ACT+DVE) of chunk i overlaps L1(i+1) matmuls.
"""

import os

import numpy as np
import ml_dtypes

import concourse.bass as bass
import concourse.mybir as mybir
import concourse.tile as tile
from concourse import bacc
from concourse import bass_utils


def _ensure_ntff_hook():
    """This image's `antenv` lacks `axon_hooks`, so boot-time NTFF hook
    install degrades silently and trace=True captures nothing. Register a
    shim module and install the ctypes-driven hook (same as trn_boot)."""
    import sys
    import types

    if "antenv.axon_hooks" in sys.modules:
        return
    mod = types.ModuleType("antenv.axon_hooks")
    mod._hook = None

    def set_axon_ntff_profile_hook(h):
        mod._hook = h

    def get_axon_ntff_profile_hook():
        return mod._hook

    mod.set_axon_ntff_profile_hook = set_axon_ntff_profile_hook
    mod.get_axon_ntff_profile_hook = get_axon_ntff_profile_hook
    sys.modules["antenv.axon_hooks"] = mod
    try:
        from trn_agent_boot.trn_boot import _ntff_profile_via_ctypes

        hook = _ntff_profile_via_ctypes("/opt/axon/libaxon_pjrt.so")
        if hook is not None:
            mod._hook = hook
    except Exception:
        pass


P = 128
D_MODEL = 2048
D_FF = 8192
N_LOCAL = 8192
N_CORES = 8
KM = D_MODEL // P              # 16 contraction tiles for layer 1
CHUNK = 512                    # d_ff chunk held in PSUM per pass
FC = CHUNK // P                # 4 d_ff tiles per chunk
NCHUNK = D_FF // CHUNK         # 16
M2 = D_MODEL // P              # 16 output d_model tiles

F32 = mybir.dt.float32
BF16 = mybir.dt.bfloat16
BF16_NP = ml_dtypes.bfloat16
GELU = mybir.ActivationFunctionType.Gelu


def _blocks(total):
    """Moving-dim blocks, each <= 512 (one PSUM bank of fp32), near-equal
    and even-sized so every block's byte offset stays 4B-aligned."""
    n = (total + 511) // 512
    pairs = total // 2
    base = pairs // n
    out = []
    off = 0
    for i in range(n):
        hp = base + (1 if i < pairs - base * n else 0)
        hs = hp * 2
        out.append((off, hs))
        off += hs
    assert off == total and all(2 <= hs <= 512 for _, hs in out)
    return out


def _build_sparse(nc, caps):
    """Per-expert gathered tokens (caps[e] per core); expert passes run
    back-to-back, with expert 1's xT prefetched during expert 0."""
    HS = [_blocks(caps[e]) for e in range(2)]
    capmax = max(caps)
    xg = [
        nc.dram_tensor(f"xg{e}", (D_MODEL, caps[e]), BF16, kind="ExternalInput").ap()
        for e in range(2)
    ]
    # w1 is host-packed to (c, k) -> contiguous [P, CHUNK] blocks so each
    # strip DMA is one linear 128KB read (column slices of the natural
    # row-major layout have 1KB lines / 16KB stride and run ~3x slower)
    w1 = [
        nc.dram_tensor(
            f"w1p_{e}", (NCHUNK * KM, P, CHUNK), BF16, kind="ExternalInput"
        ).ap()
        for e in range(2)
    ]
    w2 = [
        nc.dram_tensor(f"w2_{e}", (D_FF, D_MODEL), BF16, kind="ExternalInput").ap()
        for e in range(2)
    ]
    b1t = [
        nc.dram_tensor(f"b1t_{e}", (P, D_FF // P), F32, kind="ExternalInput").ap()
        for e in range(2)
    ]
    b2t = [
        nc.dram_tensor(f"b2t_{e}", (P, M2), F32, kind="ExternalInput").ap()
        for e in range(2)
    ]
    wgg = [
        nc.dram_tensor(f"wgg{e}", (P, caps[e]), BF16, kind="ExternalInput").ap()
        for e in range(2)
    ]
    yt = [
        nc.dram_tensor(f"yt{e}", (D_MODEL, caps[e]), F32, kind="ExternalOutput").ap()
        for e in range(2)
    ]

    with tile.TileContext(nc) as tc:
        with (
            tc.tile_pool(name="const", bufs=1) as const_pool,
            tc.tile_pool(name="w1s", bufs=8) as w1_pool,
            tc.tile_pool(name="w2s", bufs=8) as w2_pool,
            tc.tile_pool(name="ht", bufs=8) as ht_pool,
            tc.tile_pool(name="ps", bufs=8, space="PSUM") as psum_pool,
        ):
            # Both experts' xT k-tiles stay resident: expert 1's tiles
            # are prefetched while expert 0 computes.
            xt_sb = [
                [
                    const_pool.tile(
                        [P, caps[e]], BF16, tag=f"xt{e}_{k}", name=f"xt_sb{e}_{k}"
                    )
                    for k in range(KM)
                ]
                for e in range(2)
            ]
            y_sb = const_pool.tile([P, M2, capmax], F32, tag="y", name="y_sb")
            wgg_sb = [
                const_pool.tile([P, caps[e]], BF16, tag=f"wgg{e}", name=f"wgg{e}_sb")
                for e in range(2)
            ]
            b1t_sb = [
                const_pool.tile([P, D_FF // P], F32, tag=f"b1t{e}", name=f"b1t{e}_sb")
                for e in range(2)
            ]
            b2t_sb = [
                const_pool.tile([P, M2], F32, tag=f"b2t{e}", name=f"b2t{e}_sb")
                for e in range(2)
            ]

            xg3 = [xg[e].rearrange("(ko p) t -> p ko t", p=P) for e in range(2)]
            yt3 = [yt[e].rearrange("(mo p) t -> p mo t", p=P) for e in range(2)]

            pairs = [(e, c) for e in range(2) for c in range(NCHUNK)]

            def emit_l1(e, c, first=False):
                """PE: layer-1 matmuls for one (expert, chunk)."""
                cap = caps[e]
                psums = [
                    [
                        psum_pool.tile(
                            [P, hs], F32, tag="ps", name=f"ps1_{e}_{c}_{f}_{h}"
                        )
                        for h, (off, hs) in enumerate(HS[e])
                    ]
                    for f in range(FC)
                ]
                for k in range(KM):
                    if e == 0 and c == 0:
                        # Startup is bound by per-DMA-queue delivery
                        # (measured ~140 sync / ~130 gpsimd / ~50
                        # scalar GB/s); balance the 5.2MB the first
                        # chunk needs across all three so every k-tile
                        # lands just ahead of the PE's k-march. k0 is
                        # split at the h-block boundary so the very
                        # first matmul's operand arrives first.
                        if k == 0:
                            for off, hs in HS[0]:
                                nc.sync.dma_start(
                                    xt_sb[0][0][:, off : off + hs],
                                    xg3[0][:, 0, off : off + hs],
                                )
                        else:
                            nc.sync.dma_start(xt_sb[0][k][:], xg3[0][:, k, :])
                        if k == KM - 1:
                            # consts ride gpsimd after the xT tiles
                            # (nothing reads them before ~25us)
                            for ee in range(2):
                                nc.gpsimd.dma_start(wgg_sb[ee][:], wgg[ee][:])
                                nc.gpsimd.dma_start(b1t_sb[ee][:], b1t[ee][:])
                                nc.gpsimd.dma_start(b2t_sb[ee][:], b2t[ee][:])
                    if e == 0 and 6 <= c <= 13 and k in (4, 12):
                        # prefetch expert-1's xT two k-tiles per chunk.
                        # Placed on sync BETWEEN its w1 strip pushes:
                        # sync's stream is self-paced by the w1 pool's
                        # slot WARs, so these actually fire at chunk-c
                        # time instead of racing ahead into the
                        # HBM-saturated startup window (gpsimd's queue
                        # has no such pacing and blasts everything
                        # immediately).
                        kk = (c - 6) * 2 + (0 if k == 4 else 1)
                        nc.sync.dma_start(xt_sb[1][kk][:], xg3[1][:, kk, :])
                    w1s = w1_pool.tile(
                        [P, CHUNK], BF16, tag="w1s", name=f"w1s_{e}_{c}_{k}"
                    )
                    if e == 0 and c == 0:
                        # sync is dedicated to the xT stream in c0, so
                        # the w1 strips split between the two slower
                        # queues (~1MB each fits their rates). c1
                        # strips must NOT ride scalar: they'd be
                        # head-of-line blocked behind c0's ACTIVATEs
                        # in the engine FIFO.
                        w1_eng = nc.scalar if k % 2 == 0 else nc.gpsimd
                    elif e == 0 and c == 1 and k < 4:
                        # gpsimd reaches these right after its c0 work
                        # (~30us), exactly when L1(c1) needs them;
                        # sync's queue is still draining c0 then
                        w1_eng = nc.gpsimd
                    else:
                        w1_eng = nc.sync
                    w1_eng.dma_start(w1s[:], w1[e][c * KM + k])
                    for f in range(FC):
                        for h, (off, hs) in enumerate(HS[e]):
                            nc.tensor.matmul(
                                psums[f][h][:],
                                w1s[:, f * P : (f + 1) * P],
                                xt_sb[e][k][:, off : off + hs],
                                start=(k == 0),
                                stop=(k == KM - 1),
                            )
                return psums

            def emit_act(e, c, psums):
                """ACT+DVE: gelu(+b1), gate scale. Also W2 strip loads,
                and (on each expert's first chunk) the gated b2 y-init."""
                cap = caps[e]
                if c == 0:
                    for m in range(M2):
                        nc.vector.tensor_scalar_mul(
                            y_sb[:, m, :cap], wgg_sb[e][:], b2t_sb[e][:, m : m + 1]
                        )
                hts = []
                for f in range(FC):
                    ht = ht_pool.tile(
                        [P, capmax], BF16, tag="ht", name=f"ht_{e}_{c}_{f}"
                    )
                    col = c * FC + f
                    for h, (off, hs) in enumerate(HS[e]):
                        nc.scalar.activation(
                            ht[:, off : off + hs],
                            psums[f][h][:],
                            GELU,
                            bias=b1t_sb[e][:, col : col + 1],
                        )
                    nc.vector.tensor_mul(ht[:, :cap], ht[:, :cap], wgg_sb[e][:])
                    hts.append(ht)
                w2s = []
                # c0's w2 strips queue on scalar AFTER its ACTs (~40us,
                # needed ~58us) keeping them out of the contended
                # 25-35us window on sync/gpsimd
                w2_eng = nc.scalar if (e == 0 and c == 0) else nc.sync
                for f in range(FC):
                    w2f = w2_pool.tile(
                        [P, D_MODEL], BF16, tag="w2s", name=f"w2s_{e}_{c}_{f}"
                    )
                    row = (c * FC + f) * P
                    w2_eng.dma_start(w2f[:], w2[e][row : row + P, :])
                    w2s.append(w2f)
                return hts, w2s

            def emit_l2(e, c, hts, w2s):
                """PE: layer-2 matmuls; DVE: accumulate into y; store at
                the expert's last chunk."""
                cap = caps[e]
                for m in range(M2):
                    for h, (off, hs) in enumerate(HS[e]):
                        ps = psum_pool.tile(
                            [P, hs], F32, tag="ps", name=f"ps2_{e}_{c}_{m}_{h}"
                        )
                        for f in range(FC):
                            nc.tensor.matmul(
                                ps[:],
                                w2s[f][:, m * P : (m + 1) * P],
                                hts[f][:, off : off + hs],
                                start=(f == 0),
                                stop=(f == FC - 1),
                            )
                        ysl = y_sb[:, m, off : off + hs]
                        nc.vector.tensor_add(ysl, ysl, ps[:])
                    if c == NCHUNK - 1:
                        nc.sync.dma_start(yt3[e][:, m, :], y_sb[:, m, :cap])

            # PE warm-up: ~4.3us of dependency-free dummy matmuls run
            # while the startup DMAs stream in, so the HAM clock-gate
            # un-throttles (1.2 -> 2.4 GHz) before the first real
            # matmul. Without this the whole first chunk runs cold.
            # ~7us of dummy matmuls: the HAM clock-gate warms after
            # ~3.4us and stays warm (no idle gaps), while the startup
            # DMA burst (~5MB, HBM-bound) builds enough lead for chunk
            # 0 to then stream at the warm PE rate without stalls.
            warm_x = const_pool.tile([P, 512], BF16, tag="warm", name="warm_sb")
            nc.gpsimd.memset(warm_x[:], 0.0)
            for i in range(26):
                wps = psum_pool.tile([P, 512], F32, tag="ps", name=f"warm_ps{i}")
                nc.tensor.matmul(
                    wps[:], warm_x[:, :P], warm_x[:], start=True, stop=True
                )

            psums_cur = emit_l1(*pairs[0], first=True)
            for i, (e, c) in enumerate(pairs):
                hts, w2s = emit_act(e, c, psums_cur)
                if i + 1 < len(pairs):
                    psums_cur = emit_l1(*pairs[i + 1])
                emit_l2(e, c, hts, w2s)

    nc.compile()
    return nc


_CACHED = {}


def _get_nc(caps):
    key = tuple(caps)
    if key not in _CACHED:
        nc = bacc.Bacc(
            "TRN2",
            target_bir_lowering=False,
            debug=False,
            num_devices=N_CORES,
        )
        _CACHED[key] = _build_sparse(nc, caps)
    return _CACHED[key]


def _run(nc, in_maps):
    trace = bool(int(os.environ.get("KERNEL_TRACE", "0")))
    if trace:
        _ensure_ntff_hook()
    res = bass_utils.run_bass_kernel_spmd(
        nc, in_maps, core_ids=list(range(N_CORES)), trace=trace
    )
    if trace:
        kernel.last_exec_time_ns = res.exec_time_ns
        kernel.last_results = res
    return res


def kernel(**inputs):
    x = np.asarray(inputs["x_local"], dtype=np.float32)          # (8192, 2048)
    ids = np.asarray(inputs["top2_exp_id"])                       # (8192, 2)
    tw = np.asarray(inputs["top2_weight"], dtype=np.float32)      # (8192, 2)

    sel = (ids % 2).astype(np.float32)
    wge = [
        (tw * (1.0 - sel)).sum(axis=1).astype(np.float32),        # expert-0 gate
        (tw * sel).sum(axis=1).astype(np.float32),                # expert-1 gate
    ]

    xt = np.ascontiguousarray(x.T.astype(BF16_NP))                # (2048, 8192)

    shared = {}
    for e in range(2):
        w1bf = np.asarray(inputs[f"W1_{e}"], dtype=np.float32).astype(BF16_NP)
        # pack (D_MODEL, D_FF) -> (c*KM+k, P, CHUNK) contiguous blocks
        shared[f"w1p_{e}"] = np.ascontiguousarray(
            w1bf.reshape(KM, P, NCHUNK, CHUNK)
            .transpose(2, 0, 1, 3)
            .reshape(NCHUNK * KM, P, CHUNK)
        )
        shared[f"w2_{e}"] = np.ascontiguousarray(
            np.asarray(inputs[f"W2_{e}"], dtype=np.float32).astype(BF16_NP)
        )
        shared[f"b1t_{e}"] = np.ascontiguousarray(
            np.asarray(inputs[f"b1_{e}"], dtype=np.float32).reshape(D_FF // P, P).T
        )
        shared[f"b2t_{e}"] = np.ascontiguousarray(
            np.asarray(inputs[f"b2_{e}"], dtype=np.float32).reshape(M2, P).T
        )

    # Globally-balanced gathers: each expert's active set (~75% of all
    # tokens) is split evenly across the 8 cores; capacity is sized to
    # the actual max per-core load (multiple of 4), so padding waste is
    # at most 3 tokens per expert per core.
    glocs = [np.flatnonzero(wge[e] > 0) for e in range(2)]
    ceil = lambda a, b: -(-a // b)
    caps = [max(4, ceil(ceil(len(glocs[e]), N_CORES), 4) * 4) for e in range(2)]

    splits = [np.array_split(glocs[e], N_CORES) for e in range(2)]
    in_maps = []
    for c in range(N_CORES):
        m = dict(shared)
        for e in range(2):
            loc = splits[e][c]
            cnt = len(loc)
            xgc = np.zeros((D_MODEL, caps[e]), BF16_NP)
            xgc[:, :cnt] = xt[:, loc]
            m[f"xg{e}"] = xgc
            wggc = np.zeros((caps[e],), np.float32)
            wggc[:cnt] = wge[e][loc]
            m[f"wgg{e}"] = np.ascontiguousarray(
                np.broadcast_to(wggc.astype(BF16_NP), (P, caps[e]))
            )
        in_maps.append(m)

    res = _run(_get_nc(caps), in_maps)

    y = np.zeros((N_LOCAL, D_MODEL), np.float32)
    for c in range(N_CORES):
        for e in range(2):
            loc = splits[e][c]
            cnt = len(loc)
            y[loc] += res.results[c][f"yt{e}"].T[:cnt]
    return y
